# revision 28
# baseline (speedup 1.0000x reference)
"""AdaptivePyramidPool v4: window-in-partition layout, fp8 DoubleRow scores,
pair-merged elementwise ops, deferred (software-pipelined) pooling.

Token mapping per batch: t = q*1024 + p*8 + j  (p = SBUF partition, j in [0,8)).
Each partition holds 8 consecutive tokens, so every softmax window (2/4/8)
lies along the free axis -> window sums are strided DVE reduces (no PE
softmax matmuls) and x DMA packets are 24KB contiguous reads.

Per 128-token tile pair (columns j, j+1 of a q-group):
  - 12 PE transposes -> xt psum (bf16), psum->SBUF copies cast to fp8
    (split DVE/Act), 3+3 fp8 DoubleRow matmuls -> scores [128, 2, 384] psum
  - one Act tanh (pair), one gpsimd mul by v (pair), one DVE reduce -> scr
Per q-group: exp, strided window-sum reduces, reciprocal (DVE),
broadcast-muls (gpsimd) -> alpha.  Pooling (2 alpha-stationary matmuls per
tile into per-batch psum [3, 384]x2) is deferred one q-group so the PE never
stalls on the softmax chain.  After each batch: drain psum to SBUF with the
1/W scale and transpose into the fusion layout.  Tail: bf16 fusion matmul,
LayerNorm.
"""

import sys

for _p in ("/opt/pypackages", "/opt/trn_rl_repo"):
    if _p not in sys.path:
        sys.path.insert(0, _p)

from contextlib import ExitStack

import numpy as np
import ml_dtypes

import concourse.bass as bass
import concourse.tile as tile
from concourse import bacc, mybir
from concourse.bass import ts
from concourse.bass_utils import run_bass_kernel_spmd

F32 = mybir.dt.float32
BF16 = mybir.dt.bfloat16
FP8 = mybir.dt.float8e4
DR = mybir.MatmulPerfMode.DoubleRow

N_CORES = 8
POOL_SIZES = [2, 4, 8]
LN_EPS = 1e-5
J = 8  # tokens per partition-run; all pool sizes divide it


def build_nc(b_loc=4, T=4096, D=768, A=128, debug=False):
    S = 3
    DC = D // 128
    Q = T // (128 * J)
    KF = S * DC
    SA = S * A
    assert T % (128 * J) == 0 and D % 128 == 0 and A == 128

    nc = bacc.Bacc("TRN2", target_bir_lowering=False, debug=debug)

    x_d = nc.dram_tensor("x", [b_loc, T, D], F32, kind="ExternalInput")
    wp_d = nc.dram_tensor("wp_t", [128, DC * SA], BF16, kind="ExternalInput")
    v_d = nc.dram_tensor("v_t", [S, A], BF16, kind="ExternalInput")
    wf_d = nc.dram_tensor("wf_t", [128, KF * D], BF16, kind="ExternalInput")
    bf_d = nc.dram_tensor("bf", [D], F32, kind="ExternalInput")
    gam_d = nc.dram_tensor("gamma", [D], F32, kind="ExternalInput")
    bet_d = nc.dram_tensor("beta", [D], F32, kind="ExternalInput")
    out_d = nc.dram_tensor("out", [b_loc, D], F32, kind="ExternalOutput")

    ident_np = np.eye(128, dtype=ml_dtypes.bfloat16)
    id_dram = nc.inline_tensor(np.asarray(ident_np), "id_const")
    wsc_np = np.array([[p / T] for p in POOL_SIZES], dtype=np.float32)
    wsc_dram = nc.inline_tensor(wsc_np, "wsc_const")

    with tile.TileContext(nc) as tc, ExitStack() as ctx:
        singles = ctx.enter_context(tc.tile_pool(name="singles", bufs=1))
        xp = ctx.enter_context(tc.tile_pool(name="xp", bufs=4))
        xtp = ctx.enter_context(tc.tile_pool(name="xtp", bufs=3))
        mids = ctx.enter_context(tc.tile_pool(name="mids", bufs=3))
        qsm = ctx.enter_context(tc.tile_pool(name="qsm", bufs=3))
        outp = ctx.enter_context(tc.tile_pool(name="outp", bufs=2))
        ps_xt = ctx.enter_context(
            tc.tile_pool(name="ps_xt", bufs=3, space=bass.MemorySpace.PSUM))
        ps_pre = ctx.enter_context(
            tc.tile_pool(name="ps_pre", bufs=3, space=bass.MemorySpace.PSUM))
        ps_pool = ctx.enter_context(
            tc.tile_pool(name="ps_pool", bufs=1, space=bass.MemorySpace.PSUM))

        ident = singles.tile([128, 128], BF16)
        nc.sync.dma_start(out=ident, in_=id_dram[:])
        wsc_sb = singles.tile([S, 1], F32, tag="wsc")
        nc.sync.dma_start(out=wsc_sb, in_=wsc_dram[:])

        wp_bf = singles.tile([128, DC * SA], BF16, tag="wpbf")
        nc.sync.dma_start(out=wp_bf, in_=wp_d[:])
        wp8 = singles.tile([128, DC, SA], FP8, tag="wp8")
        nc.scalar.activation(out=wp8.rearrange("p c n -> p (c n)"), in_=wp_bf,
                             func=mybir.ActivationFunctionType.Copy)

        v_sb = singles.tile([128, S, A], BF16)
        v_b = bass.AP(tensor=v_d[:].tensor, offset=0,
                      ap=[[0, 128]] + v_d[:].ap)
        nc.sync.dma_start(out=v_sb, in_=v_b)
        # broadcast view of v over a tile pair: [128, 2, S, A], stride-0 pair
        v_ap = v_sb[:, :, :]
        v_pair = bass.AP(tensor=v_ap.tensor, offset=v_ap.offset,
                         ap=[v_ap.ap[0], [0, 2]] + v_ap.ap[1:])

        wf_sb = singles.tile([128, KF, D], BF16)
        bf_sb = singles.tile([b_loc, D], F32)
        gam_sb = singles.tile([b_loc, D], F32)
        bet_sb = singles.tile([b_loc, D], F32)
        eps_sb = singles.tile([b_loc, 1], F32)
        nc.vector.memset(eps_sb, LN_EPS)

        # per-batch pooled-feature psum accumulators, drained after each batch
        plo_t = ps_pool.tile([S, 384], F32, tag="plo")
        phi_t = ps_pool.tile([S, D - 384], F32, tag="phi")
        # per-batch drained features (SBUF, scaled by 1/W)
        facs = [singles.tile([S, D], BF16, tag=f"fac{b}", name=f"fac{b}")
                for b in range(b_loc)]
        # fusion operand [128, (s c), b]
        fus_sb = singles.tile([128, KF, b_loc], BF16, tag="fus")
        fus_v = fus_sb.rearrange("p (s c) b -> p c b s", s=S)

        x_v = x_d[:].rearrange("b (q p j) d -> b q p j d", q=Q, p=128, j=J)

        # deferred pooling: pooling matmuls of q-group i issue during group
        # i+2's tile loop so the PE never waits on the softmax chain, even
        # when the score reduction runs late under throttling
        pending = []

        def issue_pool(j, depth=2):
            if len(pending) < depth:
                return
            p_alpha, p_xq, p_first, p_last, p_b = pending[0]
            nc.tensor.matmul(plo_t, p_alpha[:, j, :], p_xq[:, j, 0:384],
                             start=(p_first and j == 0),
                             stop=(p_last and j == J - 1))
            nc.tensor.matmul(phi_t, p_alpha[:, j, :], p_xq[:, j, 384:D],
                             start=(p_first and j == 0),
                             stop=(p_last and j == J - 1))
            if j == J - 1:
                if p_last:
                    # batch p_b complete: drain+scale psum -> SBUF, then
                    # transpose into the fusion layout (overlaps next batch)
                    fac = facs[p_b]
                    nc.vector.tensor_scalar_mul(fac[:, 0:384], plo_t, wsc_sb)
                    nc.vector.tensor_scalar_mul(fac[:, 384:D], phi_t, wsc_sb)
                    for c in range(DC):
                        f_ps = ps_xt.tile([128, S], BF16, tag="xtps")
                        nc.tensor.transpose(f_ps, fac[:, ts(c, 128)],
                                            ident[0:S, 0:S])
                        nc.vector.tensor_copy(fus_v[:, c, p_b, :], f_ps)
                pending.pop(0)

        xqs = {}

        def fetch(bq):
            b_, q_ = divmod(bq, Q)
            if b_ >= b_loc or bq in xqs:
                return
            xq_ = xp.tile([128, J, D], BF16, name="xq")
            if bq == 0:
                nc.gpsimd.dma_start(out=xq_[:, 0:1, :],
                                    in_=x_v[0, 0, :, 0:1, :])
                nc.gpsimd.dma_start(out=xq_[:, 1:2, :],
                                    in_=x_v[0, 0, :, 1:2, :])
                nc.gpsimd.dma_start(out=xq_[:, 2:J, :],
                                    in_=x_v[0, 0, :, 2:J, :])
            else:
                nc.gpsimd.dma_start(out=xq_, in_=x_v[b_, q_])
            xqs[bq] = xq_

        fetch(0)
        for b in range(b_loc):
            for q in range(Q):
                bq = b * Q + q
                fetch(bq)
                fetch(bq + 1)
                xq = xqs.pop(bq)

                scr_q = qsm.tile([128, J, S], BF16, tag="scr")
                xt8s = {}
                e_pairs = {}

                def stage_a(j):
                    # transposes + psum->SBUF fp8 copies for tile j
                    xt_ps = ps_xt.tile([128, DC, 128], BF16, tag="xtps")
                    for c in range(DC):
                        nc.tensor.transpose(xt_ps[:, c, :],
                                            xq[:, j, ts(c, 128)], ident)
                    xt8 = xtp.tile([128, DC, 128], FP8, tag="xt8")
                    nc.vector.tensor_copy(
                        xt8[:, 0:3].rearrange("p c t -> p (c t)"),
                        xt_ps[:, 0:3].rearrange("p c t -> p (c t)"))
                    nc.scalar.activation(
                        out=xt8[:, 3:6].rearrange("p c t -> p (c t)"),
                        in_=xt_ps[:, 3:6].rearrange("p c t -> p (c t)"),
                        func=mybir.ActivationFunctionType.Copy)
                    xt8s[j] = xt8

                stage_a(0)
                stage_a(1)
                for j in range(J):
                    # attention for tile j (consumes 2-tiles-old copies)
                    xt8 = xt8s.pop(j)
                    pre = ps_pre.tile([128, SA], F32, tag="pre")
                    for g2 in range(DC // 2):
                        nc.tensor.matmul(pre,
                                         xt8[:, 2 * g2:2 * g2 + 2, :],
                                         wp8[:, 2 * g2:2 * g2 + 2, :],
                                         start=(g2 == 0),
                                         stop=(g2 == DC // 2 - 1),
                                         perf_mode=DR)
                    if j % 2 == 0:
                        e_pairs[j // 2] = mids.tile([128, 2, S, A], BF16,
                                                    tag="e", name="e_pair")
                    e_pair = e_pairs[j // 2]
                    nc.scalar.activation(
                        out=e_pair[:, j % 2].rearrange("p s a -> p (s a)"),
                        in_=pre,
                        func=mybir.ActivationFunctionType.Tanh)
                    if j + 2 < J:
                        stage_a(j + 2)
                    issue_pool(j)
                    if j % 2 == 1:
                        e_pair = e_pairs.pop(j // 2)
                        prod = mids.tile([128, 2, S, A], BF16, tag="prod")
                        nc.vector.tensor_mul(prod, e_pair, v_pair)
                        with nc.allow_low_precision(reason="bf16 scores ok"):
                            nc.vector.reduce_sum(
                                scr_q[:, j - 1:j + 1, :], prod,
                                axis=mybir.AxisListType.X)

                exps_q = qsm.tile([128, J, S], BF16, tag="exps")
                nc.scalar.activation(
                    out=exps_q.rearrange("p j s -> p (j s)"),
                    in_=scr_q.rearrange("p j s -> p (j s)"),
                    func=mybir.ActivationFunctionType.Exp)

                ws = qsm.tile([128, J], F32, tag="ws")  # 4 + 2 + 1 used
                rec = qsm.tile([128, J], F32, tag="rec")
                offs = []
                off = 0
                for s, p in enumerate(POOL_SIZES):
                    w = J // p
                    offs.append((off, w, p))
                    nc.vector.reduce_sum(
                        ws[:, off:off + w],
                        exps_q[:, :, s].rearrange("p (w i) -> p w i", i=p),
                        axis=mybir.AxisListType.X)
                    off += w
                nc.vector.reciprocal(rec[:, 0:off], ws[:, 0:off])

                alpha_q = qsm.tile([128, J, S], BF16, tag="alpha")
                for s, (o, w, p) in enumerate(offs):
                    rb = rec[:, o:o + w]
                    rb_b = bass.AP(tensor=rb.tensor, offset=rb.offset,
                                   ap=rb.ap + [[0, p]])
                    nc.vector.tensor_mul(
                        alpha_q[:, :, s].rearrange("p (w i) -> p w i", i=p),
                        exps_q[:, :, s].rearrange("p (w i) -> p w i", i=p),
                        rb_b)

                pending.append((alpha_q, xq, q == 0, q == Q - 1, b))
            if b == 1:
                # tail-only tensors; load them mid-flight to keep startup
                # DMA bandwidth for x
                nc.sync.dma_start(out=wf_sb, in_=wf_d[:])
                nc.sync.dma_start(out=bf_sb, in_=bass.AP(
                    tensor=bf_d[:].tensor, offset=0,
                    ap=[[0, b_loc]] + bf_d[:].ap))
                nc.sync.dma_start(out=gam_sb, in_=bass.AP(
                    tensor=gam_d[:].tensor, offset=0,
                    ap=[[0, b_loc]] + gam_d[:].ap))
                nc.sync.dma_start(out=bet_sb, in_=bass.AP(
                    tensor=bet_d[:].tensor, offset=0,
                    ap=[[0, b_loc]] + bet_d[:].ap))

        # flush remaining deferred pooling + drain + transposes
        while pending:
            for j in range(J):
                issue_pool(j, depth=1)

        # ---- fusion + layernorm tail ----
        ms_sb = outp.tile([b_loc, D], F32)
        stats = outp.tile([b_loc, 2, 6], F32, tag="stats")
        for h in range(2):
            ms_ps = ps_pre.tile([b_loc, D // 2], F32, tag="pre")
            for k in range(KF):
                nc.tensor.matmul(ms_ps, fus_sb[:, k, :],
                                 wf_sb[:, k, ts(h, D // 2)],
                                 start=(k == 0), stop=(k == KF - 1))
            nc.vector.tensor_add(ms_sb[:, ts(h, D // 2)], ms_ps,
                                 bf_sb[:, ts(h, D // 2)])
            nc.vector.bn_stats(stats[:, h, :], ms_sb[:, ts(h, D // 2)])
        mv = outp.tile([b_loc, 2], F32, tag="mv")
        nc.vector.bn_aggr(mv, stats)
        std = outp.tile([b_loc, 1], F32, tag="std")
        nc.scalar.activation(out=std, in_=mv[:, 1:2],
                             func=mybir.ActivationFunctionType.Sqrt,
                             bias=eps_sb)
        rstd = outp.tile([b_loc, 1], F32, tag="rstd")
        nc.vector.reciprocal(rstd, std)
        out_t = outp.tile([b_loc, D], F32, tag="out")
        nc.vector.tensor_scalar(out=out_t, in0=ms_sb,
                                scalar1=mv[:, 0:1], scalar2=rstd,
                                op0=mybir.AluOpType.subtract,
                                op1=mybir.AluOpType.mult)
        nc.vector.tensor_mul(out_t, out_t, gam_sb)
        nc.vector.tensor_add(out_t, out_t, bet_sb)
        nc.sync.dma_start(out=out_d[:], in_=out_t)

    nc.compile()
    return nc


def _prep_weights(Wp, v, Wf):
    S, D, A = Wp.shape
    DC = D // 128
    wp_t = np.ascontiguousarray(
        Wp.reshape(S, DC, 128, A).transpose(2, 1, 0, 3).reshape(128, -1)
    ).astype(ml_dtypes.bfloat16)
    v_t = np.ascontiguousarray(v).astype(ml_dtypes.bfloat16)
    wf_t = np.ascontiguousarray(
        Wf.reshape(S, DC, 128, D).transpose(2, 0, 1, 3).reshape(128, -1)
    ).astype(ml_dtypes.bfloat16)
    return wp_t, v_t, wf_t


_NC_CACHE = {}


def kernel(x, Wp, bp, v, Wf, bf, gamma, beta):
    B, T, D = x.shape
    assert B % N_CORES == 0
    b_loc = B // N_CORES
    key = (b_loc, T, D)
    if key not in _NC_CACHE:
        _NC_CACHE[key] = build_nc(b_loc=b_loc, T=T, D=D, A=Wp.shape[2])
    nc = _NC_CACHE[key]

    wp_t, v_t, wf_t = _prep_weights(
        np.asarray(Wp, np.float32), np.asarray(v, np.float32),
        np.asarray(Wf, np.float32))
    common = {
        "wp_t": wp_t,
        "v_t": v_t,
        "wf_t": wf_t,
        "bf": np.ascontiguousarray(bf, np.float32),
        "gamma": np.ascontiguousarray(gamma, np.float32),
        "beta": np.ascontiguousarray(beta, np.float32),
    }
    in_maps = [
        {"x": np.ascontiguousarray(x[i * b_loc:(i + 1) * b_loc], np.float32),
         **common}
        for i in range(N_CORES)
    ]
    res = run_bass_kernel_spmd(nc, in_maps, core_ids=list(range(N_CORES)))
    return np.concatenate([res.results[i]["out"] for i in range(N_CORES)],
                          axis=0)
